# revision 38
# baseline (speedup 1.0000x reference)
"""Window-routed sparse attention on 8 TRN2 NeuronCores.

Sharding: 64 windows x 8 cores = 8 windows/core (embarrassingly parallel).
Host precomputes the tiny routing path (region means, a_r [64,64]) and the
window-mixed q_m/k_m in fp32 numpy; each core runs the heavy windowed
attention relu(q_m k_m^T) v for its 8 windows on the Tensor engine.

v2: bf16 matmul operands (fp32r measured 4 cycles/row on HW; bf16 is 1),
relu+cast fused on alternating scalar/vector engines, fp32 PSUM accumulate.
"""

import sys

sys.path.insert(0, "/opt/trn_rl_repo")

import numpy as np
import ml_dtypes

BF16 = np.dtype(ml_dtypes.bfloat16)

C = 64          # channels
NW = 64         # windows (8x8 grid of 32x32 patches on 256x256)
T = 1024        # tokens per window (32*32)
NCORES = 8
WPC = NW // NCORES  # windows per core

_CACHE = {}

LAST_RESULT = None


def _build_program():
    import concourse.mybir as mybir
    from concourse import bacc
    from concourse.tile import TileContext

    bf16 = mybir.dt.bfloat16
    f32 = mybir.dt.float32

    nc = bacc.Bacc(None, target_bir_lowering=False)
    # c-major [c, i, t] for q_m/k_m; [i, p, k, c] for v (p=128 partition)
    qm_d = nc.declare_dram_parameter("qm", [C, WPC, T], bf16, isOutput=False)
    km_d = nc.declare_dram_parameter("km", [C, WPC, T], bf16, isOutput=False)
    v_d = nc.declare_dram_parameter("v", [WPC, 128, 8, C], bf16, isOutput=False)
    # o packed [wpc, p, u]: channel c of token half h lives at partition
    # h*64+c (PE column-tiling writes h1 to psum partitions 64-127)
    o_d = nc.declare_dram_parameter("o", [WPC, 128, 512], f32, isOutput=True)

    with TileContext(nc) as tc:
        with (
            tc.tile_pool(name="qk", bufs=2) as qk_pool,
            tc.tile_pool(name="vp", bufs=2) as v_pool,
            tc.tile_pool(name="at", bufs=3) as a_pool,
            tc.tile_pool(name="ob", bufs=2) as o_pool,
            tc.tile_pool(name="pa", bufs=3, space="PSUM") as pa_pool,
            tc.tile_pool(name="po", bufs=2, space="PSUM") as po_pool,
        ):
            # software pipeline: in step i, produce window i's relu'd attn
            # (A-phase) interleaved with window i-1's o-matmuls (B-phase).
            # B reads at_w tiles that are a full window old, so the tensor
            # stream never waits on a fresh relu.
            at_tiles = [None] * WPC
            v_tiles = [None] * WPC
            qk_tiles = [None] * WPC
            ps_o_tiles = [None] * WPC

            def emit_dma(i):
                qm_t = qk_pool.tile([C, T], bf16, tag="qm", name=f"qm{i}")
                km_t = qk_pool.tile([C, T], bf16, tag="km", name=f"km{i}")
                v_t = v_pool.tile([128, 8, C], bf16, tag="v", name=f"v{i}")
                nc.sync.dma_start(out=qm_t, in_=qm_d[:, i, :])
                nc.sync.dma_start(out=km_t, in_=km_d[:, i, :])
                nc.sync.dma_start(out=v_t, in_=v_d[i])
                qk_tiles[i] = (qm_t, km_t)
                v_tiles[i] = v_t

            def emit_a_chunk(i, k):
                if k == 0:
                    at_tiles[i] = a_pool.tile(
                        [128, 8, T], bf16, tag="attn", name=f"at{i}"
                    )
                qm_t, km_t = qk_tiles[i]
                at_w = at_tiles[i]
                ps_a = pa_pool.tile([128, T], f32, tag="psa")
                for h in range(2):
                    nc.tensor.matmul(
                        out=ps_a[:, h * 512:(h + 1) * 512],
                        lhsT=km_t[:, k * 128:(k + 1) * 128],
                        rhs=qm_t[:, h * 512:(h + 1) * 512],
                        start=True,
                        stop=True,
                    )
                # scalar ACT is ~20% faster than DVE tensor_scalar: give it 5
                # of 8 chunks; vector also owns the ps_o copies.
                if k in (0, 2, 4, 6, 7):
                    nc.scalar.activation(
                        out=at_w[:, k, :],
                        in_=ps_a,
                        func=mybir.ActivationFunctionType.Relu,
                        scale=1.0,
                    )
                else:
                    nc.vector.tensor_scalar_max(at_w[:, k, :], ps_a, 0.0)

            def emit_b_chunk(i, k, hs=(0, 1)):
                if k == 0 and 0 in hs:
                    ps_o_tiles[i] = po_pool.tile(
                        [128, 512], f32, tag="pso", name=f"pso{i}"
                    )
                ps_o = ps_o_tiles[i]
                at_w = at_tiles[i]
                v_t = v_tiles[i]
                for h in hs:
                    nc.tensor.matmul(
                        out=ps_o[h * 64:(h + 1) * 64, :],
                        lhsT=v_t[:, k, :],
                        rhs=at_w[:, k, h * 512:(h + 1) * 512],
                        start=(k == 0),
                        stop=(k == 7),
                    )

            def emit_copy(w):
                ps_o = ps_o_tiles[w]
                o_t = o_pool.tile([128, 512], f32, tag="o", name=f"o{w}")
                nc.vector.tensor_copy(out=o_t, in_=ps_o)
                nc.sync.dma_start(out=o_d[w], in_=o_t)

            # per window: A block (qk matmuls, relus drain to SBUF), then B
            # block (o matmuls). The ps_o->SBUF copy of window w is deferred
            # into mid-A(w+1) so it never blocks fresh relus in the in-order
            # ALU queues; input DMAs prefetch one window ahead.
            emit_dma(0)
            for step in range(WPC):
                for k in range(8):
                    emit_a_chunk(step, k)
                    if k == 1 and step + 1 < WPC:
                        emit_dma(step + 1)
                    if k == 3 and step >= 1:
                        emit_copy(step - 1)
                last = step == WPC - 1
                if not last:
                    for k in range(8):
                        emit_b_chunk(step, k)
                else:
                    # h-major so the h0 half of ps_o completes early; copy and
                    # DMA it while the h1 matmuls still run (shorter tail).
                    w = step
                    for k in range(8):
                        emit_b_chunk(w, k, hs=(0,))
                    o_t = o_pool.tile([128, 512], f32, tag="o", name=f"o{w}")
                    nc.vector.tensor_copy(
                        out=o_t[0:64, :], in_=ps_o_tiles[w][0:64, :]
                    )
                    nc.sync.dma_start(out=o_d[w, 0:64, :], in_=o_t[0:64, :])
                    for k in range(8):
                        emit_b_chunk(w, k, hs=(1,))
                    nc.scalar.copy(
                        out=o_t[64:128, :], in_=ps_o_tiles[w][64:128, :]
                    )
                    nc.sync.dma_start(out=o_d[w, 64:128, :], in_=o_t[64:128, :])


    nc.finalize()
    return nc


def kernel(x, W, bias):
    import os
    from concourse.bass_utils import run_bass_kernel_spmd

    x = np.asarray(x, dtype=np.float32)
    W = np.asarray(W, dtype=np.float32)
    bias = np.asarray(bias, dtype=np.float32)

    # ---- host prep: windows, qkv, routing, mixing (tiny vs attention) ----
    # xw: [nw, T, c]
    xw = (
        x.reshape(C, 8, 32, 8, 32)
        .transpose(1, 3, 2, 4, 0)
        .reshape(NW, T, C)
    )
    qkv = xw @ W.T + bias  # [nw, T, 3c]
    q, k, v = qkv[..., :C], qkv[..., C:2 * C], qkv[..., 2 * C:]
    q_r = q.mean(axis=1)  # [nw, c]
    k_r = k.mean(axis=1)
    a_r = np.maximum(q_r @ k_r.T, 0.0)  # [nw, nw]
    k_m = np.tensordot(a_r, k, axes=(1, 0))  # [nw, T, c]
    q_m = np.tensordot(a_r, q, axes=(1, 0))

    if "nc" not in _CACHE:
        _CACHE["nc"] = _build_program()
    nc = _CACHE["nc"]

    # [nw, 128, 8, c]: s-chunk-of-128 minor-major layout for the o matmul
    v_dev = np.ascontiguousarray(
        v.reshape(NW, 8, 128, C).transpose(0, 2, 1, 3)
    ).astype(BF16)
    qm_dev = np.ascontiguousarray(q_m.transpose(2, 0, 1)).astype(BF16)  # [c,nw,T]
    km_dev = np.ascontiguousarray(k_m.transpose(2, 0, 1)).astype(BF16)

    in_maps = []
    for m in range(NCORES):
        s = slice(m * WPC, (m + 1) * WPC)
        in_maps.append({
            "qm": qm_dev[:, s, :],
            "km": km_dev[:, s, :],
            "v": v_dev[s],
        })

    trace = bool(os.environ.get("KERNEL_TRACE"))
    res = run_bass_kernel_spmd(nc, in_maps, list(range(NCORES)), trace=trace)
    global LAST_RESULT
    LAST_RESULT = res
    # unpack [wpc, 2, 64, 512] -> [c, wpc, T]
    outs = [
        res.results[m]["o"]
        .reshape(WPC, 2, C, 512)
        .transpose(2, 0, 1, 3)
        .reshape(C, WPC, T)
        for m in range(NCORES)
    ]
    o_cm = np.concatenate(outs, axis=1)  # [c, nw, T]

    # fold back: [c, jh, jw, th, tw] -> [1, c, 256, 256]
    o_img = (
        o_cm.reshape(C, 8, 8, 32, 32)
        .transpose(0, 1, 3, 2, 4)
        .reshape(1, C, 256, 256)
    )
    return o_img.astype(np.float32)


# revision 45
# speedup vs baseline: 1.1113x; 1.1113x over previous
"""Window-routed sparse attention on 8 TRN2 NeuronCores.

Sharding: 64 windows x 8 cores = 8 windows/core (embarrassingly parallel).
Host precomputes the tiny routing path (region means, a_r [64,64]) and the
window-mixed q_m/k_m in fp32 numpy; each core runs the heavy windowed
attention relu(q_m k_m^T) v for its 8 windows on the Tensor engine.

v2: bf16 matmul operands (fp32r measured 4 cycles/row on HW; bf16 is 1),
relu+cast fused on alternating scalar/vector engines, fp32 PSUM accumulate.
"""

import sys

sys.path.insert(0, "/opt/trn_rl_repo")

import numpy as np
import ml_dtypes

BF16 = np.dtype(ml_dtypes.bfloat16)

C = 64          # channels
NW = 64         # windows (8x8 grid of 32x32 patches on 256x256)
T = 1024        # tokens per window (32*32)
NCORES = 8
WPC = NW // NCORES  # windows per core

_CACHE = {}

LAST_RESULT = None


def _build_program():
    import concourse.mybir as mybir
    from concourse import bacc
    from concourse.tile import TileContext

    bf16 = mybir.dt.bfloat16
    f32 = mybir.dt.float32

    nc = bacc.Bacc(None, target_bir_lowering=False)
    # c-major [c, i, t] for q_m/k_m; [i, p, k, c] for v (p=128 partition)
    qm_d = nc.declare_dram_parameter("qm", [C, WPC, T], bf16, isOutput=False)
    km_d = nc.declare_dram_parameter("km", [C, WPC, T], bf16, isOutput=False)
    v_d = nc.declare_dram_parameter("v", [WPC, 128, 8, C], bf16, isOutput=False)
    o_d = nc.declare_dram_parameter("o", [C, WPC, T], f32, isOutput=True)

    with TileContext(nc) as tc:
        with (
            tc.tile_pool(name="qk", bufs=2) as qk_pool,
            tc.tile_pool(name="vp", bufs=2) as v_pool,
            tc.tile_pool(name="at", bufs=3) as a_pool,
            tc.tile_pool(name="ob", bufs=2) as o_pool,
            tc.tile_pool(name="pa", bufs=3, space="PSUM") as pa_pool,
            tc.tile_pool(name="po", bufs=1, space="PSUM") as po_pool,
        ):
            # software pipeline: in step i, produce window i's relu'd attn
            # (A-phase) interleaved with window i-1's o-matmuls (B-phase).
            # B reads at_w tiles that are a full window old, so the tensor
            # stream never waits on a fresh relu.
            at_tiles = [None] * WPC
            v_tiles = [None] * WPC
            qk_tiles = [None] * WPC
            ps_o_tiles = [None] * WPC

            def emit_dma(i):
                qm_t = qk_pool.tile([C, T], bf16, tag="qm", name=f"qm{i}")
                km_t = qk_pool.tile([C, T], bf16, tag="km", name=f"km{i}")
                v_t = v_pool.tile([128, 8, C], bf16, tag="v", name=f"v{i}")
                nc.sync.dma_start(out=qm_t, in_=qm_d[:, i, :])
                nc.sync.dma_start(out=km_t, in_=km_d[:, i, :])
                nc.sync.dma_start(out=v_t, in_=v_d[i])
                qk_tiles[i] = (qm_t, km_t)
                v_tiles[i] = v_t

            def emit_a_chunk(i, k):
                if k == 0:
                    at_tiles[i] = a_pool.tile(
                        [128, 8, T], bf16, tag="attn", name=f"at{i}"
                    )
                qm_t, km_t = qk_tiles[i]
                at_w = at_tiles[i]
                ps_a = pa_pool.tile([128, T], f32, tag="psa")
                for h in range(2):
                    nc.tensor.matmul(
                        out=ps_a[:, h * 512:(h + 1) * 512],
                        lhsT=km_t[:, k * 128:(k + 1) * 128],
                        rhs=qm_t[:, h * 512:(h + 1) * 512],
                        start=True,
                        stop=True,
                    )
                # scalar ACT is ~20% faster than DVE tensor_scalar: give it 5
                # of 8 chunks; vector also owns the ps_o copies.
                if k in (0, 2, 4, 6, 7):
                    nc.scalar.activation(
                        out=at_w[:, k, :],
                        in_=ps_a,
                        func=mybir.ActivationFunctionType.Relu,
                        scale=1.0,
                    )
                else:
                    nc.vector.tensor_scalar_max(at_w[:, k, :], ps_a, 0.0)

            def emit_b_chunk(i, k, hs=(0, 1)):
                if k == 0 and 0 in hs:
                    ps_o_tiles[i] = po_pool.tile(
                        [C, T], f32, tag="pso", name=f"pso{i}"
                    )
                ps_o = ps_o_tiles[i]
                at_w = at_tiles[i]
                v_t = v_tiles[i]
                for h in hs:
                    nc.tensor.matmul(
                        out=ps_o[:, h * 512:(h + 1) * 512],
                        lhsT=v_t[:, k, :],
                        rhs=at_w[:, k, h * 512:(h + 1) * 512],
                        start=(k == 0),
                        stop=(k == 7),
                    )

            def emit_copy(w):
                ps_o = ps_o_tiles[w]
                o_t = o_pool.tile([C, T], f32, tag="o", name=f"o{w}")
                nc.vector.tensor_copy(out=o_t, in_=ps_o)
                nc.sync.dma_start(out=o_d[:, w, :], in_=o_t)

            # per window: A block (qk matmuls, relus drain to SBUF), then B
            # block (o matmuls). The ps_o->SBUF copy of window w is deferred
            # into mid-A(w+1) so it never blocks fresh relus in the in-order
            # ALU queues; input DMAs prefetch one window ahead.
            def emit_a_phase(i, ks):
                for k in ks:
                    emit_a_chunk(i, k)
                    if k == 1 and i + 1 < WPC:
                        emit_dma(i + 1)
                    if k == 3 and i >= 1:
                        emit_copy(i - 1)

            emit_dma(0)
            emit_a_phase(0, range(8))
            for step in range(WPC - 1):
                # partial shift: the first two chunks of A(step+1) run before
                # B(step), giving the freshest relus ~1.7us extra headroom
                # before their o-matmuls arrive.
                emit_a_phase(step + 1, range(2))
                for k in range(8):
                    emit_b_chunk(step, k)
                emit_a_phase(step + 1, range(2, 8))
            # last window: h-major so the h0 half of ps_o completes early;
            # copy and DMA it while the h1 matmuls still run (shorter tail).
            w = WPC - 1
            for k in range(8):
                emit_b_chunk(w, k, hs=(0,))
            o_t = o_pool.tile([C, T], f32, tag="o", name=f"o{w}")
            nc.vector.tensor_copy(
                out=o_t[:, 0:512], in_=ps_o_tiles[w][:, 0:512]
            )
            nc.sync.dma_start(out=o_d[:, w, 0:512], in_=o_t[:, 0:512])
            for k in range(8):
                emit_b_chunk(w, k, hs=(1,))
            nc.scalar.copy(
                out=o_t[:, 512:1024], in_=ps_o_tiles[w][:, 512:1024]
            )
            nc.sync.dma_start(out=o_d[:, w, 512:1024], in_=o_t[:, 512:1024])


    nc.finalize()
    return nc


def kernel(x, W, bias):
    import os
    from concourse.bass_utils import run_bass_kernel_spmd

    x = np.asarray(x, dtype=np.float32)
    W = np.asarray(W, dtype=np.float32)
    bias = np.asarray(bias, dtype=np.float32)

    # ---- host prep: windows, qkv, routing, mixing (tiny vs attention) ----
    # xw: [nw, T, c]
    xw = (
        x.reshape(C, 8, 32, 8, 32)
        .transpose(1, 3, 2, 4, 0)
        .reshape(NW, T, C)
    )
    qkv = xw @ W.T + bias  # [nw, T, 3c]
    q, k, v = qkv[..., :C], qkv[..., C:2 * C], qkv[..., 2 * C:]
    q_r = q.mean(axis=1)  # [nw, c]
    k_r = k.mean(axis=1)
    a_r = np.maximum(q_r @ k_r.T, 0.0)  # [nw, nw]
    k_m = np.tensordot(a_r, k, axes=(1, 0))  # [nw, T, c]
    q_m = np.tensordot(a_r, q, axes=(1, 0))

    if "nc" not in _CACHE:
        _CACHE["nc"] = _build_program()
    nc = _CACHE["nc"]

    # [nw, 128, 8, c]: s-chunk-of-128 minor-major layout for the o matmul
    v_dev = np.ascontiguousarray(
        v.reshape(NW, 8, 128, C).transpose(0, 2, 1, 3)
    ).astype(BF16)
    qm_dev = np.ascontiguousarray(q_m.transpose(2, 0, 1)).astype(BF16)  # [c,nw,T]
    km_dev = np.ascontiguousarray(k_m.transpose(2, 0, 1)).astype(BF16)

    in_maps = []
    for m in range(NCORES):
        s = slice(m * WPC, (m + 1) * WPC)
        in_maps.append({
            "qm": qm_dev[:, s, :],
            "km": km_dev[:, s, :],
            "v": v_dev[s],
        })

    trace = bool(os.environ.get("KERNEL_TRACE"))
    res = run_bass_kernel_spmd(nc, in_maps, list(range(NCORES)), trace=trace)
    global LAST_RESULT
    LAST_RESULT = res
    outs = [res.results[m]["o"].reshape(C, WPC, T) for m in range(NCORES)]
    o_cm = np.concatenate(outs, axis=1)  # [c, nw, T]

    # fold back: [c, jh, jw, th, tw] -> [1, c, 256, 256]
    o_img = (
        o_cm.reshape(C, 8, 8, 32, 32)
        .transpose(0, 1, 3, 2, 4)
        .reshape(1, C, 256, 256)
    )
    return o_img.astype(np.float32)
